# revision 1
# baseline (speedup 1.0000x reference)
"""Trainium2 Bass kernel for nn_EqvMSPFeedForward (continuous-filter conv + scatter-sum).

Math refactoring vs the reference:
  reference:  K = radial_mlp(r) @ w3            # [B,N,N,C*C] = 536 MB materialized
              y = einsum('zaboi,zbi->zao', K.reshape(...,C,C), x)
  here:       the einsum is linear in w3, so contract x into w3 first:
              G[b,h,o] = sum_i w3[h, o*C+i] * x[b,i]        (per batch)
              y[a,o]   = sum_b sum_h h2[h, pair(a,b)] * G[b,h,o]
  which never materializes K (0.8 GFLOP total instead of ~27 GFLOP and 536 MB).

Sharding: data-parallel over batch B=8 across the 8 NeuronCores; each core
computes one batch element end-to-end (no collectives).
"""

import math
import sys

sys.path.insert(0, "/opt/trn_rl_repo")

import numpy as np

import concourse.bass as bass
import concourse.tile as tile
from concourse import bacc, mybir
from concourse.bass_utils import run_bass_kernel_spmd

# problem constants (hardcoded per contract)
B, N1, N2, C = 8, 32, 32, 64
N = N1 + N2                    # 64 positions
NP = N * N                     # 4096 pairs
NB = 10                        # number of radial basis functions
H = 100                        # radial MLP hidden width
MAX_RADIUS = 10.0
STEP = MAX_RADIUS / (NB - 1)
SBASIS = (math.pi / 2.0) / STEP
HALF_PI = math.pi / 2.0
SCALE1 = 1.0 / math.sqrt(NB)   # fan-in norm after basis @ w1
SCALE2 = 1.0 / math.sqrt(H)    # after h1 @ w2 and h2 @ w3
SCALE_FINAL = SCALE2 / math.sqrt(N)   # w3 fan-in * 1/sqrt(N) on the einsum
FC_SCALE = 1.0 / math.sqrt(C)
LRELU_ALPHA = 0.01
EPS = 1e-6

F32 = mybir.dt.float32
F16 = mybir.dt.float16
F32R = mybir.dt.float32r
I32 = mybir.dt.int32

N_CORES = 8
PAIR_CHUNK = 512               # pairs per MLP matmul (one PSUM bank)
N_CHUNKS = NP // PAIR_CHUNK    # 8


def _build_nc(stage: str = "full") -> bass.Bass:
    nc = bacc.Bacc("TRN2", target_bir_lowering=False)

    # ---- DRAM I/O (per-core shapes: one batch element) ----
    d_xT2 = nc.dram_tensor("xT2", [128, 128], F16, kind="ExternalInput")
    d_xyzT = nc.dram_tensor("xyzT", [3, N], F32, kind="ExternalInput")
    d_mask = nc.dram_tensor("mask", [N, 1], I32, kind="ExternalInput")
    d_w1 = nc.dram_tensor("w1", [NB, H], F16, kind="ExternalInput")
    d_w2 = nc.dram_tensor("w2", [H, H], F16, kind="ExternalInput")
    d_w3T = nc.dram_tensor("w3T", [128, (C * C // 128) * H], F16, kind="ExternalInput")
    d_fc3 = nc.dram_tensor("fc3_w", [C, C], F32, kind="ExternalInput")
    d_fc2 = nc.dram_tensor("fc2_w", [C, 1], F32, kind="ExternalInput")
    out_shape = {
        "r": [N, N], "basis": [NB, NP], "h2": [H, NP], "G": [H, C * N],
        "y": [N, C], "full": [1, 1],
    }[stage]
    d_out = nc.dram_tensor("out", out_shape, F32, kind="ExternalOutput")

    # ---- inline constants ----
    radii_np = np.arange(NB, dtype=np.float32) * STEP
    delta = (HALF_PI / SBASIS) * (1.0 - 1e-5)  # keep fp sin arg inside [0, pi]
    d_bconst = nc.inline_tensor(
        np.stack([radii_np - delta, radii_np + delta,
                  HALF_PI - SBASIS * radii_np], axis=1).astype(np.float32),
        name="bconst")

    def _emit(tc, single, work, ps_h1, ps_h2, ps_g, ps_fin, ps_small):
        # ---------- constant / weight loads ----------
        # (sync issues only the latency-critical small loads; weights go out
        # on the Activation HWDGE, w3T on the GpSimd SWDGE)
        bconst = single.tile([NB, 3], F32)
        nc.sync.dma_start(out=bconst, in_=d_bconst[:, :])
        rlo, rhi, sbias = bconst[:, 0:1], bconst[:, 1:2], bconst[:, 2:3]

        w1_sb = single.tile([NB, H], F16)
        nc.scalar.dma_start(out=w1_sb, in_=d_w1[:, :])
        w2_sb = single.tile([H, H], F16)
        nc.scalar.dma_start(out=w2_sb, in_=d_w2[:, :])

        # mask -> float 0/1 column [N,1]
        mask_i = single.tile([N, 1], I32)
        nc.sync.dma_start(out=mask_i, in_=d_mask[:, :])
        mask_f = single.tile([N, 1], F32)
        nc.vector.tensor_copy(out=mask_f, in_=mask_i)
        nc.vector.tensor_scalar(
            out=mask_f, in0=mask_f, scalar1=0.0, scalar2=None,
            op0=mybir.AluOpType.not_equal,
        )

        # ---------- pair distances r[a,b] ----------
        # r2 = |xyz_a|^2 - 2 xyz_a . xyz_b + |xyz_b|^2 via one K=5 matmul:
        # aug_l rows: [-2*xyzT (3), nrow (1), ones (1)]
        # aug_r rows: [xyzT (3), ones (1), nrow (1)]
        # compute engines must start at partition 0/32/64/96, so rows 3:5
        # of the aug tiles are filled via SBUF->SBUF DMA.
        aug_r = single.tile([5, N], F32)
        nc.sync.dma_start(out=aug_r[0:3, :], in_=d_xyzT[:, :])

        ones_row = single.tile([1, N], F32)
        nc.vector.memset(ones_row, 1.0)

        xyzT2 = work.tile([3, N], F32)
        nc.vector.tensor_mul(out=xyzT2, in0=aug_r[0:3, :], in1=aug_r[0:3, :])
        ones3 = single.tile([3, 1], F32)
        nc.vector.memset(ones3, 1.0)
        ps_nrow = ps_small.tile([1, N], F32, tag="sm")
        nc.tensor.matmul(out=ps_nrow, lhsT=ones3, rhs=xyzT2, start=True, stop=True)
        nrow_sb = work.tile([1, N], F32)
        nc.vector.tensor_copy(out=nrow_sb, in_=ps_nrow)

        aug_l = single.tile([5, N], F32)
        nc.scalar.mul(out=aug_l[0:3, :], in_=aug_r[0:3, :], mul=-2.0)
        nc.sync.dma_start(out=aug_r[3:4, :], in_=ones_row)
        nc.sync.dma_start(out=aug_r[4:5, :], in_=nrow_sb)
        nc.sync.dma_start(out=aug_l[3:4, :], in_=nrow_sb)
        nc.sync.dma_start(out=aug_l[4:5, :], in_=ones_row)

        # big loads AFTER the small latency-critical ones above
        # Block-diagonal [[xT, 0], [0, xT]] (host-prepared) with
        # xT[i, b] = x[b, i]: one K=128 matmul against a full w3T chunk
        # computes two o-planes of G (the zero blocks kill cross terms).
        xT2 = single.tile([128, 128], F16)
        nc.scalar.dma_start(out=xT2, in_=d_xT2[:, :])
        # w3T (host-pretransposed): w3T[p, c, h] = w3[h, c*128 + p]; chunked
        # DMAs so G matmuls can start before the full 1.6 MB lands.
        w3T_sb = single.tile([128, C * C // 128, H], F16)
        for j in range(8):
            eng = (nc.sync, nc.sync, nc.sync, nc.scalar, nc.scalar, nc.scalar,
                   nc.gpsimd, nc.gpsimd)[j]
            eng.dma_start(
                out=w3T_sb[:, 4 * j : 4 * j + 4, :],
                in_=d_w3T[:, j * 4 * H : (j + 1) * 4 * H],
            )

        ps_r2 = ps_small.tile([N, N], F32, tag="sm")
        nc.tensor.matmul(out=ps_r2, lhsT=aug_l, rhs=aug_r, start=True, stop=True)

        # clamp >= 0 (cancellation can leave tiny negatives on the diagonal)
        r2_sb = work.tile([N, N], F32)
        nc.vector.tensor_scalar(
            out=r2_sb, in0=ps_r2, scalar1=0.0, scalar2=None,
            op0=mybir.AluOpType.max,
        )
        eps12 = single.tile([N, 1], F32)
        nc.vector.memset(eps12, 1e-12)
        r_sb = work.tile([N, N], F32)
        nc.scalar.activation(
            out=r_sb, in_=r2_sb, func=mybir.ActivationFunctionType.Sqrt,
            bias=eps12,
        )

        if stage == "r":
            nc.sync.dma_start(out=d_out[:, :], in_=r_sb)
        # ---------- radial basis [NB, NP] ----------
        do_basis = stage != "r" 
        # flatten r to [1, NP], broadcast to [NB, NP]
        if not do_basis:
            return
        r_flat = work.tile([1, NP], F32)
        nc.sync.dma_start(out=r_flat, in_=r_sb[:, :])
        r10 = single.tile([NB, NP], F32)
        nc.sync.dma_start(
            out=r10,
            in_=bass.AP(tensor=r_flat.tensor, offset=r_flat.offset,
                        ap=[[1, 1], [0, NB], [1, NP]]),
        )

        # ---------- fused basis + radial MLP, chunk-pipelined ----------
        # clamp r to [radii_k - delta, radii_k + delta]; then
        # cos((r - radii_k)*SBASIS) = sin(SBASIS*rc + pi/2 - SBASIS*radii_k),
        # argument in [0, pi] (Sin valid range); basis = cos^2.
        basis = single.tile([NB, NP], F32)
        basis_r = single.tile([NB, NP], F16)
        h2_sb = single.tile([H, NP], F16)
        nc.vector.tensor_scalar(
            out=basis, in0=r10, scalar1=rlo, scalar2=rhi,
            op0=mybir.AluOpType.max, op1=mybir.AluOpType.min,
        )
        nc.scalar.activation(
            out=basis, in_=basis, func=mybir.ActivationFunctionType.Sin,
            scale=SBASIS, bias=sbias,
        )
        for j in (range(N_CHUNKS) if stage != "G" else []):
            sl = slice(j * PAIR_CHUNK, (j + 1) * PAIR_CHUNK)
            nc.vector.tensor_mul(
                out=basis_r[:, sl], in0=basis[:, sl], in1=basis[:, sl])
            p1 = ps_h1.tile([H, PAIR_CHUNK], F32)
            nc.tensor.matmul(out=p1, lhsT=w1_sb, rhs=basis_r[:, sl], start=True, stop=True)
            h1c = work.tile([H, PAIR_CHUNK], F16)
            nc.scalar.activation(
                out=h1c, in_=p1, func=mybir.ActivationFunctionType.Silu,
                scale=SCALE1,
            )
            p2 = ps_h2.tile([H, PAIR_CHUNK], F32)
            nc.tensor.matmul(out=p2, lhsT=w2_sb, rhs=h1c, start=True, stop=True)
            nc.scalar.activation(
                out=h2_sb[:, sl], in_=p2, func=mybir.ActivationFunctionType.Silu,
                scale=SCALE2,
            )
        if stage == "h2":
            nc.sync.dma_start(out=d_out[:, :], in_=h2_sb)
            return

        # ---------- G[h, o*N + b] = sum_i w3[h, o*C+i] * x[b, i] ----------
        # One matmul per w3T chunk: lhsT [128=(2 o's, i), H], rhs = xT2
        # block-diag -> out [H, 128] = [G_{2c} | G_{2c+1}].
        G_sb = single.tile([H, C * N], F16)
        for c in range(32):
            pg = ps_g.tile([H, 128], F32, tag="g")
            nc.tensor.matmul(
                out=pg, lhsT=w3T_sb[:, c, :], rhs=xT2, start=True, stop=True,
            )
            nc.vector.tensor_copy(out=G_sb[:, c * 128 : (c + 1) * 128], in_=pg)
        if stage == "G":
            nc.sync.dma_start(out=d_out[:, :], in_=G_sb)
            return

        fc3_sb = single.tile([C, C], F32)
        nc.gpsimd.dma_start(out=fc3_sb, in_=d_fc3[:, :])
        fc2_sb = single.tile([C, 1], F32)
        nc.gpsimd.dma_start(out=fc2_sb, in_=d_fc2[:, :])

        # ---------- final contraction: y[a, o] = sum_b h2_b^T @ G_b ----------
        # h2[:, b*N:(b+1)*N] works as h2[h, a*N+b] by (a,b) symmetry of r.
        G_v = G_sb[:, :].rearrange("h (o b) -> h b o", b=N)
        ps_y = ps_fin.tile([N, C], F32)
        for b in range(N):
            nc.tensor.matmul(
                out=ps_y,
                lhsT=h2_sb[:, b * N : (b + 1) * N],
                rhs=G_v[:, b, :],
                start=(b == 0), stop=(b == N - 1),
            )

        # ---------- |y| * mask, column-sum over a ----------
        absx = work.tile([N, C], F32)
        nc.scalar.activation(
            out=absx, in_=ps_y, func=mybir.ActivationFunctionType.Abs,
            scale=SCALE_FINAL,
        )
        if stage == "y":
            nc.sync.dma_start(out=d_out[:, :], in_=absx)
            return
        nc.vector.tensor_scalar_mul(out=absx, in0=absx, scalar1=mask_f)

        ones64 = single.tile([N, 1], F32)
        nc.vector.memset(ones64, 1.0)
        ps_s = ps_small.tile([1, C], F32, tag="sm")
        nc.tensor.matmul(out=ps_s, lhsT=ones64, rhs=absx, start=True, stop=True)

        # ---------- mean/std(ddof=1) normalize over C ----------
        ssum = work.tile([1, 1], F32)
        nc.vector.reduce_sum(out=ssum, in_=ps_s, axis=mybir.AxisListType.X)
        m_s = work.tile([1, 1], F32)
        nc.vector.tensor_scalar_mul(out=m_s, in0=ssum, scalar1=1.0 / C)
        d_row = work.tile([1, C], F32)
        nc.vector.tensor_scalar(
            out=d_row, in0=ps_s, scalar1=m_s, scalar2=None,
            op0=mybir.AluOpType.subtract,
        )
        dsq = work.tile([1, C], F32)
        nc.vector.tensor_mul(out=dsq, in0=d_row, in1=d_row)
        qsum = work.tile([1, 1], F32)
        nc.vector.reduce_sum(out=qsum, in_=dsq, axis=mybir.AxisListType.X)
        stddev = work.tile([1, 1], F32)
        nc.scalar.activation(
            out=stddev, in_=qsum, func=mybir.ActivationFunctionType.Sqrt,
            scale=1.0 / (C - 1),
        )
        nc.vector.tensor_scalar_add(out=stddev, in0=stddev, scalar1=EPS)
        rec = work.tile([1, 1], F32)
        nc.vector.reciprocal(out=rec, in_=stddev)
        norm_row = work.tile([1, C], F32)
        nc.vector.tensor_scalar_mul(out=norm_row, in0=d_row, scalar1=rec)

        # ---------- head: leaky_relu(norm @ fc3 / 8) @ fc2 / 8 -> sigmoid ----------
        ident1 = single.tile([1, 1], F32)
        nc.vector.memset(ident1, 1.0)
        ps_nT = ps_small.tile([C, 1], F32, tag="sm")
        nc.tensor.transpose(out=ps_nT, in_=norm_row, identity=ident1)
        normT = work.tile([C, 1], F32)
        nc.vector.tensor_copy(out=normT, in_=ps_nT)

        ps_y1 = ps_small.tile([C, 1], F32, tag="sm")
        nc.tensor.matmul(out=ps_y1, lhsT=fc3_sb, rhs=normT, start=True, stop=True)
        y1_sb = work.tile([C, 1], F32)
        y1_neg = work.tile([C, 1], F32)
        nc.vector.tensor_scalar_mul(out=y1_sb, in0=ps_y1, scalar1=FC_SCALE)
        nc.vector.tensor_scalar_mul(out=y1_neg, in0=ps_y1, scalar1=FC_SCALE * LRELU_ALPHA)
        nc.vector.tensor_tensor(
            out=y1_sb, in0=y1_sb, in1=y1_neg, op=mybir.AluOpType.max)

        ps_y2 = ps_small.tile([1, 1], F32, tag="sm")
        nc.tensor.matmul(out=ps_y2, lhsT=y1_sb, rhs=fc2_sb, start=True, stop=True)
        res = work.tile([1, 1], F32)
        nc.scalar.activation(
            out=res, in_=ps_y2, func=mybir.ActivationFunctionType.Sigmoid,
            scale=FC_SCALE,
        )
        nc.sync.dma_start(out=d_out[:, :], in_=res)

    with tile.TileContext(nc) as tc:
        with (
            tc.tile_pool(name="single", bufs=1) as single,
            tc.tile_pool(name="work", bufs=2) as work,
            tc.tile_pool(name="ps_h1", bufs=2, space="PSUM") as ps_h1,
            tc.tile_pool(name="ps_h2", bufs=2, space="PSUM") as ps_h2,
            tc.tile_pool(name="ps_g", bufs=2, space="PSUM") as ps_g,
            tc.tile_pool(name="ps_fin", bufs=1, space="PSUM") as ps_fin,
            tc.tile_pool(name="ps_small", bufs=1, space="PSUM") as ps_small,
        ):
            _emit(tc, single, work, ps_h1, ps_h2, ps_g, ps_fin, ps_small)
    nc.finalize()
    return nc


_NC_CACHE = None


def _get_nc():
    global _NC_CACHE
    if _NC_CACHE is None:
        _NC_CACHE = _build_nc()
    return _NC_CACHE


def kernel(**inputs) -> np.ndarray:
    nc = _get_nc()
    # pure relayout of w3 (done once on host): w3T[p, c, h] = w3[h, c*128+p]
    w3 = np.asarray(inputs["w3"], dtype=np.float32)
    w3T = np.ascontiguousarray(
        w3.reshape(H, C * C // 128, 128).transpose(2, 1, 0).reshape(128, -1)
    ).astype(np.float16)
    in_maps = []
    for z in range(N_CORES):
        xT = np.concatenate(
            [inputs["input1"][z], inputs["input2"][z]], axis=0).astype(np.float32).T
        xT2 = np.zeros((128, 128), np.float16)
        xT2[0:64, 0:64] = xT
        xT2[64:128, 64:128] = xT
        in_maps.append({
            "xT2": xT2,
            "xyzT": np.ascontiguousarray(np.concatenate(
                [inputs["xyz1"][z], inputs["xyz2"][z]], axis=0).astype(np.float32).T),
            "mask": np.ascontiguousarray(
                inputs["mask"][z].reshape(N, 1), dtype=np.int32),
            "w1": np.ascontiguousarray(inputs["w1"], dtype=np.float16),
            "w2": np.ascontiguousarray(inputs["w2"], dtype=np.float16),
            "w3T": w3T,
            "fc3_w": np.ascontiguousarray(inputs["fc3_w"], dtype=np.float32),
            "fc2_w": np.ascontiguousarray(inputs["fc2_w"], dtype=np.float32),
        })
    out = run_bass_kernel_spmd(nc, in_maps, core_ids=list(range(N_CORES)))
    return np.concatenate(
        [r["out"].reshape(-1) for r in out.results]).astype(np.float32)



# revision 20
# speedup vs baseline: 1.2390x; 1.2390x over previous
"""Trainium2 Bass kernel for nn_EqvMSPFeedForward (continuous-filter conv + scatter-sum).

Math refactoring vs the reference:
  reference:  K = radial_mlp(r) @ w3            # [B,N,N,C*C] = 536 MB materialized
              y = einsum('zaboi,zbi->zao', K.reshape(...,C,C), x)
  here:       the einsum is linear in w3, so contract x into w3 first:
              G[b,h,o] = sum_i w3[h, o*C+i] * x[b,i]        (per batch)
              y[a,o]   = sum_b sum_h h2[h, pair(a,b)] * G[b,h,o]
  which never materializes K.

Sharding: data-parallel over batch B=8 across the 8 NeuronCores; each core
computes one batch element end-to-end (no collectives).

Layout notes vs the previous version of this kernel:
  - pair distances r^2 via PSUM accumulation of two matmuls (Gram + ones x n_row)
    with the -2 scale and +n_a folded into the Sqrt activation -- no SBUF->SBUF
    aug-tile assembly.
  - the radial basis runs in a dense quad layout [128, 1024]: partition 32*q+k
    holds pair-quarter q with basis index k, so clamp/sin/square use all four
    SBUF quadrants instead of 10 partitions.  r is replicated into that layout
    with a 3-deep DMA doubling tree (reads spread over partitions; the old
    single strided broadcast DMA ran at 2.8 GB/s due to partition port
    contention).
  - the final contraction over (h, b) uses DoubleRow matmuls: 2 b-planes per
    matmul via [K, 2, M] access patterns.
  - scalar-engine activation tables (Sqrt/Sin/Silu) are preloaded by dummy
    activations while other engines run, so the 1.28us table swaps stay off the
    critical path.
"""

import math
import sys

sys.path.insert(0, "/opt/trn_rl_repo")

import numpy as np

import concourse.bass as bass
import concourse.tile as tile
from concourse import bacc, mybir
from concourse.bass_utils import run_bass_kernel_spmd

# problem constants (hardcoded per contract)
B, N1, N2, C = 8, 32, 32, 64
N = N1 + N2                    # 64 positions
NP = N * N                     # 4096 pairs
NB = 10                        # number of radial basis functions
H = 100                        # radial MLP hidden width
MAX_RADIUS = 10.0
STEP = MAX_RADIUS / (NB - 1)
SBASIS = (math.pi / 2.0) / STEP
HALF_PI = math.pi / 2.0
SCALE1 = 1.0 / math.sqrt(NB)   # fan-in norm after basis @ w1
SCALE2 = 1.0 / math.sqrt(H)    # after h1 @ w2 and h2 @ w3
SCALE_FINAL = SCALE2 / math.sqrt(N)   # w3 fan-in * 1/sqrt(N) on the einsum
FC_SCALE = 1.0 / math.sqrt(C)
LRELU_ALPHA = 0.01
EPS = 1e-6

F32 = mybir.dt.float32
F16 = mybir.dt.float16
I32 = mybir.dt.int32

N_CORES = 8
PAIR_CHUNK = 512               # pairs per MLP matmul (one PSUM bank)
N_CHUNKS = NP // PAIR_CHUNK    # 8
QCOL = 1024                    # pair-quarter width in the quad basis layout


def _build_nc(stage: str = "full") -> bass.Bass:
    nc = bacc.Bacc("TRN2", target_bir_lowering=False)

    # ---- DRAM I/O (per-core shapes: one batch element) ----
    d_xT2 = nc.dram_tensor("xT2", [128, 128], F16, kind="ExternalInput")
    d_xyzT = nc.dram_tensor("xyzT", [3, N], F32, kind="ExternalInput")
    d_xyz = nc.dram_tensor("xyz", [N, 3], F32, kind="ExternalInput")
    d_mask = nc.dram_tensor("mask", [N, 1], I32, kind="ExternalInput")
    d_w1 = nc.dram_tensor("w1", [128, H], F16, kind="ExternalInput")
    d_w2 = nc.dram_tensor("w2", [H, H], F16, kind="ExternalInput")
    d_w3T = nc.dram_tensor("w3T", [128, (C * C // 128) * H], F16, kind="ExternalInput")
    d_fc3 = nc.dram_tensor("fc3_w", [C, C], F32, kind="ExternalInput")
    d_fc2 = nc.dram_tensor("fc2_w", [C, 1], F32, kind="ExternalInput")
    out_shape = {
        "r": [N, N], "basis": [128, QCOL], "h2": [H, NP], "G": [H, C * N],
        "y": [N, C], "full": [1, 1],
    }[stage]
    d_out = nc.dram_tensor("out", out_shape, F32, kind="ExternalOutput")
    d_rscr = nc.dram_tensor("rscr", [N, N], F32, kind="Internal")

    # ---- inline constants: per-partition (rlo, rhi, sin bias) in quad layout
    radii_np = np.arange(NB, dtype=np.float32) * STEP
    delta = (HALF_PI / SBASIS) * (1.0 - 1e-5)  # keep fp sin arg inside [0, pi]
    bc_small = np.stack(
        [radii_np - delta, radii_np + delta, HALF_PI - SBASIS * radii_np],
        axis=1).astype(np.float32)            # [NB, 3]
    bc_quad = np.zeros((128, 3), np.float32)
    for q in range(4):
        bc_quad[32 * q : 32 * q + NB, :] = bc_small
    d_bconst = nc.inline_tensor(bc_quad, name="bconst")

    def _emit(tc, single, work, ps_h1, ps_h2, ps_g, ps_fin, ps_small):
        AF = mybir.ActivationFunctionType
        OP = mybir.AluOpType

        # ---------- scalar activation-table preload (Sqrt) ----------
        dumin = single.tile([1, 1], F32)
        nc.vector.memset(dumin, 1.0)
        dum = single.tile([1, 1], F32)
        nc.scalar.activation(out=dum, in_=dumin, func=AF.Sqrt)

        # ---------- critical small loads first, each on its own queue ----------
        xyzT = single.tile([3, N], F32)
        nc.sync.dma_start(out=xyzT, in_=d_xyzT[:, :])
        xyz = single.tile([N, 3], F32)
        nc.scalar.dma_start(out=xyz, in_=d_xyz[:, :])

        xT2 = single.tile([128, 128], F16)
        nc.gpsimd.dma_start(out=xT2, in_=d_xT2[:, :])

        w1_sb = single.tile([128, H], F16)
        nc.scalar.dma_start(out=w1_sb, in_=d_w1[:, :])
        w2_sb = single.tile([H, H], F16)
        nc.scalar.dma_start(out=w2_sb, in_=d_w2[:, :])

        # w3T (host-pretransposed): w3T[p, c, h] = w3[h, c*128 + p]; eight
        # 100 KB chunks, earliest-needed first on the gpsimd queue (sync's
        # queue stays clean for the r broadcast tree).
        w3T_sb = single.tile([128, C * C // 128, H], F16)
        w3_engs = (nc.gpsimd, nc.gpsimd, nc.gpsimd, nc.gpsimd,
                   nc.scalar, nc.gpsimd, nc.scalar, nc.gpsimd)
        for j in range(8):
            w3_engs[j].dma_start(
                out=w3T_sb[:, 4 * j : 4 * j + 4, :],
                in_=d_w3T[:, j * 4 * H : (j + 1) * 4 * H],
            )
        bconst = single.tile([128, 3], F32)
        nc.gpsimd.dma_start(out=bconst, in_=d_bconst[:, :])
        rlo, rhi, sbias = bconst[:, 0:1], bconst[:, 1:2], bconst[:, 2:3]
        mask_i = single.tile([N, 1], I32)
        nc.gpsimd.dma_start(out=mask_i, in_=d_mask[:, :])
        fc3_sb = single.tile([C, C], F32)
        nc.gpsimd.dma_start(out=fc3_sb, in_=d_fc3[:, :])
        fc2_sb = single.tile([C, 1], F32)
        nc.gpsimd.dma_start(out=fc2_sb, in_=d_fc2[:, :])

        # mask -> float 0/1 column [N,1] (gpsimd: off the critical engines)
        mask_f32 = single.tile([N, 1], F32)
        nc.vector.tensor_copy(out=mask_f32, in_=mask_i)
        mask_f = single.tile([N, 1], F32)
        nc.vector.tensor_scalar(
            out=mask_f, in0=mask_f32, scalar1=0.0, scalar2=None,
            op0=OP.not_equal,
        )

        # ---------- pair distances ----------
        # PSUM r2acc = xyz_a . xyz_b - n_b/2   (Gram mm + (-1/2 ones) x n_row mm)
        # then r = Sqrt(-2 * min(r2acc, n_a/2) + (n_a + 1e-12))
        xyzT2 = work.tile([3, N], F32)
        nc.vector.tensor_mul(out=xyzT2, in0=xyzT, in1=xyzT)
        xyzsq = work.tile([N, 3], F32)
        nc.vector.tensor_mul(out=xyzsq, in0=xyz, in1=xyz)
        ncol = single.tile([N, 1], F32)
        nc.vector.reduce_sum(out=ncol, in_=xyzsq, axis=mybir.AxisListType.X)
        ncol_eps = single.tile([N, 1], F32)
        nc.vector.tensor_scalar_add(out=ncol_eps, in0=ncol, scalar1=1e-12)
        nhalf = single.tile([N, 1], F32)
        nc.vector.tensor_scalar_mul(out=nhalf, in0=ncol, scalar1=0.5)

        ones3 = single.tile([3, 1], F32)
        nc.vector.memset(ones3, 1.0)
        neghalf = single.tile([1, N], F32)
        nc.vector.memset(neghalf, -0.5)

        ps_nrow = ps_g.tile([1, N], F32, tag="g")
        nc.tensor.matmul(out=ps_nrow, lhsT=ones3, rhs=xyzT2, start=True, stop=True)
        nrow_sb = work.tile([1, N], F32)
        nc.vector.tensor_copy(out=nrow_sb, in_=ps_nrow)

        ps_r2 = ps_small.tile([N, N], F32, tag="sm")
        nc.tensor.matmul(out=ps_r2, lhsT=xyzT, rhs=xyzT, start=True, stop=False)
        nc.tensor.matmul(out=ps_r2, lhsT=neghalf, rhs=nrow_sb, start=False, stop=True)

        r2m = work.tile([N, N], F32)
        nc.vector.tensor_scalar(
            out=r2m, in0=ps_r2, scalar1=nhalf, scalar2=None, op0=OP.min,
        )
        r_sb = work.tile([N, N], F32)
        nc.scalar.activation(
            out=r_sb, in_=r2m, func=AF.Sqrt, bias=ncol_eps, scale=-2.0,
        )
        if stage == "r":
            nc.sync.dma_start(out=d_out[:, :], in_=r_sb)
            return
        # preload Sin table while the broadcast tree runs
        nc.scalar.activation(out=dum, in_=dumin, func=AF.Sin)

        # ---------- broadcast r into quad layout Q[32q+k, :] = r-quarter q ----
        # D1: two copies of r (a-major quarters) to rows {32q+0, 32q+1}
        # D2: rows {0,1} -> {2,3};  D3a: {0..3} -> {4..7};  D3b: {0,1} -> {8,9}
        Q = single.tile([128, QCOL], F32)

        def qap(row0, nrep, ncols=QCOL):
            dims = [[32, 4], [1, ncols]]
            if nrep > 1:
                dims = [[1, nrep]] + dims
            return bass.AP(
                tensor=Q.tensor, offset=Q[row0 : row0 + 1, :].offset,
                ap=dims,
            )

        nc.sync.dma_start(out=d_rscr[:, :], in_=r_sb[:, :])
        c_engs = (nc.sync, nc.scalar, nc.sync, nc.scalar)
        for q in range(4):
            qsrc = bass.AP(
                tensor=d_rscr, offset=QCOL * q,
                ap=[[1, 1], [0, NB], [1, QCOL]],
            )
            c_engs[q].dma_start(out=Q[32 * q : 32 * q + NB, :], in_=qsrc)

        # ---------- dense radial basis: clamp -> sin -> square ----------
        Qc = single.tile([128, QCOL], F32)
        nc.vector.tensor_scalar(
            out=Qc, in0=Q, scalar1=rlo, scalar2=rhi, op0=OP.max, op1=OP.min,
        )
        Qs = single.tile([128, QCOL], F16)
        nc.scalar.activation(
            out=Qs, in_=Qc, func=AF.Sin, scale=SBASIS, bias=sbias,
        )
        # preload Silu table while gpsimd squares the basis
        nc.scalar.activation(out=dum, in_=dumin, func=AF.Silu)
        Qsq = single.tile([128, QCOL], F16)
        nc.vector.tensor_mul(out=Qsq, in0=Qs, in1=Qs)
        if stage == "basis":
            nc.sync.dma_start(out=d_out[:, :], in_=Qsq)
            return

        # ---------- G[h, o*N + b] = sum_i w3[h, o*C+i] * x[b, i] ----------
        # One matmul per w3T chunk: lhsT [128=(2 o's, i), H], rhs = xT2
        # block-diag -> out [H, 128] = [G_{2c} | G_{2c+1}].
        G_sb = single.tile([H, C * N], F16)

        def emit_g(g):
            # four G matmuls into quarters of one PSUM bank (start=True zeroes
            # the whole 2KB zero-region; the rest accumulate onto pending
            # zeros), then a single PSUM->SBUF copy for all 512 cols.
            pg = ps_g.tile([H, 512], F32, tag="g")
            for t in range(4):
                c = 4 * g + t
                nc.tensor.matmul(
                    out=pg[:, 128 * t : 128 * (t + 1)],
                    lhsT=w3T_sb[:, c, :], rhs=xT2,
                    start=(t == 0), stop=(t == 3), skip_group_check=True,
                )
            nc.vector.tensor_copy(
                out=G_sb[:, 512 * g : 512 * (g + 1)], in_=pg)

        for g in range(4):
            emit_g(g)
        if stage == "G":
            for g in range(4, 8):
                emit_g(g)
            nc.sync.dma_start(out=d_out[:, :], in_=G_sb)
            return

        # ---------- fused radial MLP, chunk-pipelined, G interleaved ----------
        h2_sb = single.tile([H, NP], F16)
        ps_y = ps_fin.tile([N, C], F32, tag="y")
        G_v = G_sb[:, :].rearrange("h (o b) -> h b o", b=N)
        ydone = [0]

        def emit_y(b_hi):
            # y[a,o] += h2_b^T @ G_b ; h2[:, b*N:(b+1)*N] works as h2[h, a*N+b]
            # by (a,b) symmetry of r.  rhs[k, n] = G[k, 64*n + b].
            while ydone[0] < b_hi:
                b = ydone[0]
                nc.tensor.matmul(
                    out=ps_y,
                    lhsT=h2_sb[:, b * N : (b + 1) * N],
                    rhs=G_v[:, b, :],
                    start=(b == 0), stop=(b == N - 1),
                )
                ydone[0] += 1

        for j in range(N_CHUNKS if stage != "y" else 0):
            q, half = j // 2, j % 2
            p1 = ps_h1.tile([H, PAIR_CHUNK], F32)
            nc.tensor.matmul(
                out=p1, lhsT=w1_sb[32 * q : 32 * q + NB, :],
                rhs=Qsq[32 * q : 32 * q + NB,
                        half * PAIR_CHUNK : (half + 1) * PAIR_CHUNK],
                start=True, stop=True,
                tile_position=(32 * q, 0),
            )
            h1c = work.tile([H, PAIR_CHUNK], F16)
            nc.scalar.activation(out=h1c, in_=p1, func=AF.Silu, scale=SCALE1)
            p2 = ps_h2.tile([H, PAIR_CHUNK], F32)
            nc.tensor.matmul(out=p2, lhsT=w2_sb, rhs=h1c, start=True, stop=True)
            nc.scalar.activation(
                out=h2_sb[:, j * PAIR_CHUNK : (j + 1) * PAIR_CHUNK],
                in_=p2, func=AF.Silu, scale=SCALE2,
            )
            if j < 4:
                emit_g(4 + j)
            else:
                # all G in SBUF; contract h2 chunks as they appear
                emit_y(8 * j)
        if stage == "h2":
            nc.sync.dma_start(out=d_out[:, :], in_=h2_sb)
            return
        emit_y(N)
        # preload Sqrt table (for the std) behind the tail y matmuls
        nc.scalar.activation(out=dum, in_=dumin, func=AF.Sqrt)

        # ---------- |y| * mask, column-sum over a ----------
        absx = work.tile([N, C], F16)
        nc.scalar.activation(
            out=absx, in_=ps_y, func=AF.Abs, scale=SCALE_FINAL,
        )
        if stage == "y":
            nc.sync.dma_start(out=d_out[:, :], in_=absx)
            return
        nc.vector.tensor_scalar_mul(out=absx, in0=absx, scalar1=mask_f)

        ones64 = single.tile([N, 1], F16)
        nc.vector.memset(ones64, 1.0)
        ps_s = ps_small.tile([1, C], F32, tag="sm")
        nc.tensor.matmul(out=ps_s, lhsT=ones64, rhs=absx, start=True, stop=True)

        # ---------- mean/std(ddof=1) normalize over C ----------
        ssum = work.tile([1, 1], F32)
        nc.vector.reduce_sum(out=ssum, in_=ps_s, axis=mybir.AxisListType.X)
        m_s = work.tile([1, 1], F32)
        nc.vector.tensor_scalar_mul(out=m_s, in0=ssum, scalar1=1.0 / C)
        d_row = work.tile([1, C], F32)
        nc.vector.tensor_scalar(
            out=d_row, in0=ps_s, scalar1=m_s, scalar2=None, op0=OP.subtract,
        )
        dsq = work.tile([1, C], F32)
        nc.vector.tensor_mul(out=dsq, in0=d_row, in1=d_row)
        qsum = work.tile([1, 1], F32)
        nc.vector.reduce_sum(out=qsum, in_=dsq, axis=mybir.AxisListType.X)
        stddev = work.tile([1, 1], F32)
        nc.scalar.activation(
            out=stddev, in_=qsum, func=AF.Sqrt, scale=1.0 / (C - 1),
        )
        nc.vector.tensor_scalar_add(out=stddev, in0=stddev, scalar1=EPS)
        rec = work.tile([1, 1], F32)
        nc.vector.reciprocal(out=rec, in_=stddev)
        norm_row = work.tile([1, C], F32)
        nc.vector.tensor_scalar_mul(out=norm_row, in0=d_row, scalar1=rec)

        # ---------- head: leaky_relu(norm @ fc3 / 8) @ fc2 / 8 -> sigmoid ----
        ident1 = single.tile([1, 1], F32)
        nc.vector.memset(ident1, 1.0)
        ps_nT = ps_small.tile([C, 1], F32, tag="sm")
        nc.tensor.transpose(out=ps_nT, in_=norm_row, identity=ident1)
        normT = work.tile([C, 1], F32)
        nc.vector.tensor_copy(out=normT, in_=ps_nT)

        ps_y1 = ps_small.tile([C, 1], F32, tag="sm")
        nc.tensor.matmul(out=ps_y1, lhsT=fc3_sb, rhs=normT, start=True, stop=True)
        y1_sb = work.tile([C, 1], F32)
        y1_neg = work.tile([C, 1], F32)
        nc.vector.tensor_scalar_mul(out=y1_sb, in0=ps_y1, scalar1=FC_SCALE)
        nc.vector.tensor_scalar_mul(
            out=y1_neg, in0=ps_y1, scalar1=FC_SCALE * LRELU_ALPHA)
        nc.vector.tensor_tensor(out=y1_sb, in0=y1_sb, in1=y1_neg, op=OP.max)

        ps_y2 = ps_small.tile([1, 1], F32, tag="sm")
        nc.tensor.matmul(out=ps_y2, lhsT=y1_sb, rhs=fc2_sb, start=True, stop=True)
        res = work.tile([1, 1], F32)
        nc.scalar.activation(
            out=res, in_=ps_y2, func=AF.Sigmoid, scale=FC_SCALE,
        )
        nc.sync.dma_start(out=d_out[:, :], in_=res)

    with tile.TileContext(nc) as tc:
        with (
            tc.tile_pool(name="single", bufs=1) as single,
            tc.tile_pool(name="work", bufs=2) as work,
            tc.tile_pool(name="ps_h1", bufs=2, space="PSUM") as ps_h1,
            tc.tile_pool(name="ps_h2", bufs=2, space="PSUM") as ps_h2,
            tc.tile_pool(name="ps_g", bufs=2, space="PSUM") as ps_g,
            tc.tile_pool(name="ps_fin", bufs=1, space="PSUM") as ps_fin,
            tc.tile_pool(name="ps_small", bufs=1, space="PSUM") as ps_small,
        ):
            _emit(tc, single, work, ps_h1, ps_h2, ps_g, ps_fin, ps_small)
    nc.finalize()
    return nc


_NC_CACHE = None


def _get_nc():
    global _NC_CACHE
    if _NC_CACHE is None:
        _NC_CACHE = _build_nc()
    return _NC_CACHE


def kernel(**inputs) -> np.ndarray:
    nc = _get_nc()
    # pure relayout of w3 (done once on host): w3T[p, c, h] = w3[h, c*128+p]
    w3 = np.asarray(inputs["w3"], dtype=np.float32)
    w3T = np.ascontiguousarray(
        w3.reshape(H, C * C // 128, 128).transpose(2, 1, 0).reshape(128, -1)
    ).astype(np.float16)
    w1q = np.zeros((128, H), np.float16)
    for q in range(4):
        w1q[32 * q : 32 * q + NB, :] = np.asarray(inputs["w1"], np.float16)
    in_maps = []
    for z in range(N_CORES):
        xT = np.concatenate(
            [inputs["input1"][z], inputs["input2"][z]], axis=0).astype(np.float32).T
        xT2 = np.zeros((128, 128), np.float16)
        xT2[0:64, 0:64] = xT
        xT2[64:128, 64:128] = xT
        xyz = np.concatenate(
            [inputs["xyz1"][z], inputs["xyz2"][z]], axis=0).astype(np.float32)
        in_maps.append({
            "xT2": xT2,
            "xyzT": np.ascontiguousarray(xyz.T),
            "xyz": np.ascontiguousarray(xyz),
            "mask": np.ascontiguousarray(
                inputs["mask"][z].reshape(N, 1), dtype=np.int32),
            "w1": w1q,
            "w2": np.ascontiguousarray(inputs["w2"], dtype=np.float16),
            "w3T": w3T,
            "fc3_w": np.ascontiguousarray(inputs["fc3_w"], dtype=np.float32),
            "fc2_w": np.ascontiguousarray(inputs["fc2_w"], dtype=np.float32),
        })
    out = run_bass_kernel_spmd(nc, in_maps, core_ids=list(range(N_CORES)))
    return np.concatenate(
        [r["out"].reshape(-1) for r in out.results]).astype(np.float32)


# revision 22
# speedup vs baseline: 1.2796x; 1.0328x over previous
"""Trainium2 Bass kernel for nn_EqvMSPFeedForward (continuous-filter conv + scatter-sum).

Math refactoring vs the reference:
  reference:  K = radial_mlp(r) @ w3            # [B,N,N,C*C] = 536 MB materialized
              y = einsum('zaboi,zbi->zao', K.reshape(...,C,C), x)
  here:       the einsum is linear in w3, so contract x into w3 first:
              G[b,h,o] = sum_i w3[h, o*C+i] * x[b,i]        (per batch)
              y[a,o]   = sum_b sum_h h2[h, pair(a,b)] * G[b,h,o]
  which never materializes K.

Sharding: data-parallel over batch B=8 across the 8 NeuronCores; each core
computes one batch element end-to-end (no collectives).

Layout notes vs the previous version of this kernel:
  - pair distances r^2 via PSUM accumulation of two matmuls (Gram + ones x n_row)
    with the -2 scale and +n_a folded into the Sqrt activation -- no SBUF->SBUF
    aug-tile assembly.
  - the radial basis runs in a dense quad layout [128, 1024]: partition 32*q+k
    holds pair-quarter q with basis index k, so clamp/sin/square use all four
    SBUF quadrants instead of 10 partitions.  r is replicated into that layout
    with a 3-deep DMA doubling tree (reads spread over partitions; the old
    single strided broadcast DMA ran at 2.8 GB/s due to partition port
    contention).
  - the final contraction over (h, b) uses DoubleRow matmuls: 2 b-planes per
    matmul via [K, 2, M] access patterns.
  - scalar-engine activation tables (Sqrt/Sin/Silu) are preloaded by dummy
    activations while other engines run, so the 1.28us table swaps stay off the
    critical path.
"""

import math
import sys

sys.path.insert(0, "/opt/trn_rl_repo")

import numpy as np

import concourse.bass as bass
import concourse.tile as tile
from concourse import bacc, mybir
from concourse.bass_utils import run_bass_kernel_spmd

# problem constants (hardcoded per contract)
B, N1, N2, C = 8, 32, 32, 64
N = N1 + N2                    # 64 positions
NP = N * N                     # 4096 pairs
NB = 10                        # number of radial basis functions
H = 100                        # radial MLP hidden width
MAX_RADIUS = 10.0
STEP = MAX_RADIUS / (NB - 1)
SBASIS = (math.pi / 2.0) / STEP
HALF_PI = math.pi / 2.0
SCALE1 = 1.0 / math.sqrt(NB)   # fan-in norm after basis @ w1
SCALE2 = 1.0 / math.sqrt(H)    # after h1 @ w2 and h2 @ w3
SCALE_FINAL = SCALE2 / math.sqrt(N)   # w3 fan-in * 1/sqrt(N) on the einsum
FC_SCALE = 1.0 / math.sqrt(C)
LRELU_ALPHA = 0.01
EPS = 1e-6

F32 = mybir.dt.float32
F16 = mybir.dt.float16
I32 = mybir.dt.int32

N_CORES = 8
PAIR_CHUNK = 512               # pairs per MLP matmul (one PSUM bank)
N_CHUNKS = NP // PAIR_CHUNK    # 8
QCOL = 1024                    # pair-quarter width in the quad basis layout


def _build_nc(stage: str = "full") -> bass.Bass:
    nc = bacc.Bacc("TRN2", target_bir_lowering=False)

    # ---- DRAM I/O (per-core shapes: one batch element) ----
    d_xT2 = nc.dram_tensor("xT2", [128, 128], F16, kind="ExternalInput")
    d_xyzT = nc.dram_tensor("xyzT", [3, N], F32, kind="ExternalInput")
    d_xyz = nc.dram_tensor("xyz", [N, 3], F32, kind="ExternalInput")
    d_mask = nc.dram_tensor("mask", [N, 1], I32, kind="ExternalInput")
    d_w1 = nc.dram_tensor("w1", [128, H], F16, kind="ExternalInput")
    d_w2 = nc.dram_tensor("w2", [H, H], F16, kind="ExternalInput")
    d_w3T = nc.dram_tensor("w3T", [128, (C * C // 128) * H], F16, kind="ExternalInput")
    d_fc3 = nc.dram_tensor("fc3_w", [C, C], F32, kind="ExternalInput")
    d_fc2 = nc.dram_tensor("fc2_w", [C, 1], F32, kind="ExternalInput")
    out_shape = {
        "r": [N, N], "basis": [128, QCOL], "h2": [H, NP], "G": [H, C * N],
        "y": [N, C], "full": [1, 1],
    }[stage]
    d_out = nc.dram_tensor("out", out_shape, F32, kind="ExternalOutput")
    d_rscr = nc.dram_tensor("rscr", [N, N], F32, kind="Internal")

    # ---- inline constants: per-partition (rlo, rhi, sin bias) in quad layout
    radii_np = np.arange(NB, dtype=np.float32) * STEP
    delta = (HALF_PI / SBASIS) * (1.0 - 1e-5)  # keep fp sin arg inside [0, pi]
    bc_small = np.stack(
        [radii_np - delta, radii_np + delta, HALF_PI - SBASIS * radii_np],
        axis=1).astype(np.float32)            # [NB, 3]
    bc_quad = np.zeros((128, 3), np.float32)
    for q in range(4):
        bc_quad[32 * q : 32 * q + NB, :] = bc_small
    d_bconst = nc.inline_tensor(bc_quad, name="bconst")

    def _emit(tc, single, work, ps_h1, ps_h2, ps_g, ps_fin, ps_small):
        AF = mybir.ActivationFunctionType
        OP = mybir.AluOpType

        # ---------- scalar activation-table preload (Sqrt) ----------
        dumin = single.tile([1, 1], F32)
        nc.vector.memset(dumin, 1.0)
        dum = single.tile([1, 1], F32)
        nc.scalar.activation(out=dum, in_=dumin, func=AF.Sqrt)

        # ---------- critical small loads first, each on its own queue ----------
        xyzT = single.tile([3, N], F32)
        nc.sync.dma_start(out=xyzT, in_=d_xyzT[:, :])
        mask_i = single.tile([N, 1], I32)
        nc.sync.dma_start(out=mask_i, in_=d_mask[:, :])
        bconst = single.tile([128, 3], F32)
        nc.sync.dma_start(out=bconst, in_=d_bconst[:, :])
        rlo, rhi, sbias = bconst[:, 0:1], bconst[:, 1:2], bconst[:, 2:3]
        xyz = single.tile([N, 3], F32)
        nc.scalar.dma_start(out=xyz, in_=d_xyz[:, :])

        xT2 = single.tile([128, 128], F16)
        nc.gpsimd.dma_start(out=xT2, in_=d_xT2[:, :])

        w1_sb = single.tile([128, H], F16)
        nc.scalar.dma_start(out=w1_sb, in_=d_w1[:, :])
        w2_sb = single.tile([H, H], F16)
        nc.scalar.dma_start(out=w2_sb, in_=d_w2[:, :])

        # w3T (host-pretransposed): w3T[p, c, h] = w3[h, c*128 + p]; eight
        # 100 KB chunks, earliest-needed first on the gpsimd queue (sync's
        # queue stays clean for the r broadcast tree).
        w3T_sb = single.tile([128, C * C // 128, H], F16)
        w3_engs = (nc.gpsimd, nc.gpsimd, nc.gpsimd, nc.gpsimd,
                   nc.scalar, nc.gpsimd, nc.scalar, nc.gpsimd)
        for j in range(8):
            w3_engs[j].dma_start(
                out=w3T_sb[:, 4 * j : 4 * j + 4, :],
                in_=d_w3T[:, j * 4 * H : (j + 1) * 4 * H],
            )
        fc3_sb = single.tile([C, C], F32)
        nc.gpsimd.dma_start(out=fc3_sb, in_=d_fc3[:, :])
        fc2_sb = single.tile([C, 1], F32)
        nc.gpsimd.dma_start(out=fc2_sb, in_=d_fc2[:, :])

        # ---------- pair distances ----------
        # PSUM r2acc = xyz_a . xyz_b - n_b/2   (Gram mm + (-1/2 ones) x n_row mm)
        # then r = Sqrt(-2 * min(r2acc, n_a/2) + (n_a + 1e-12))
        xyzT2 = work.tile([3, N], F32)
        nc.vector.tensor_mul(out=xyzT2, in0=xyzT, in1=xyzT)
        xyzsq = work.tile([N, 3], F32)
        nc.vector.tensor_mul(out=xyzsq, in0=xyz, in1=xyz)
        ncol = single.tile([N, 1], F32)
        nc.vector.reduce_sum(out=ncol, in_=xyzsq, axis=mybir.AxisListType.X)
        ncol_eps = single.tile([N, 1], F32)
        nc.vector.tensor_scalar_add(out=ncol_eps, in0=ncol, scalar1=1e-12)
        nhalf = single.tile([N, 1], F32)
        nc.vector.tensor_scalar_mul(out=nhalf, in0=ncol, scalar1=0.5)

        ones3 = single.tile([3, 1], F32)
        nc.vector.memset(ones3, 1.0)
        neghalf = single.tile([1, N], F32)
        nc.vector.memset(neghalf, -0.5)

        ps_nrow = ps_g.tile([1, N], F32, tag="g")
        nc.tensor.matmul(out=ps_nrow, lhsT=ones3, rhs=xyzT2, start=True, stop=True)
        nrow_sb = work.tile([1, N], F32)
        nc.vector.tensor_copy(out=nrow_sb, in_=ps_nrow)

        ps_r2 = ps_small.tile([N, N], F32, tag="sm")
        nc.tensor.matmul(out=ps_r2, lhsT=xyzT, rhs=xyzT, start=True, stop=False)
        nc.tensor.matmul(out=ps_r2, lhsT=neghalf, rhs=nrow_sb, start=False, stop=True)

        r2m = work.tile([N, N], F32)
        nc.vector.tensor_scalar(
            out=r2m, in0=ps_r2, scalar1=nhalf, scalar2=None, op0=OP.min,
        )
        r_sb = work.tile([N, N], F32)
        nc.scalar.activation(
            out=r_sb, in_=r2m, func=AF.Sqrt, bias=ncol_eps, scale=-2.0,
        )
        if stage == "r":
            nc.sync.dma_start(out=d_out[:, :], in_=r_sb)
            return
        # mask -> float 0/1 column [N,1] (emitted late: keeps the in-order
        # vector stream from stalling the r path on the mask DMA)
        mask_f32 = single.tile([N, 1], F32)
        nc.vector.tensor_copy(out=mask_f32, in_=mask_i)
        mask_f = single.tile([N, 1], F32)
        nc.vector.tensor_scalar(
            out=mask_f, in0=mask_f32, scalar1=0.0, scalar2=None,
            op0=OP.not_equal,
        )
        # preload Sin table while the broadcast tree runs (dep on r_sb pins
        # this after the sqrt in the schedule)
        nc.scalar.activation(out=dum, in_=r_sb[0:1, 0:1], func=AF.Sin)

        # ---------- broadcast r into quad layout Q[32q+k, :] = r-quarter q ----
        # D1: two copies of r (a-major quarters) to rows {32q+0, 32q+1}
        # D2: rows {0,1} -> {2,3};  D3a: {0..3} -> {4..7};  D3b: {0,1} -> {8,9}
        Q = single.tile([128, QCOL], F32)

        def qap(row0, nrep, ncols=QCOL):
            dims = [[32, 4], [1, ncols]]
            if nrep > 1:
                dims = [[1, nrep]] + dims
            return bass.AP(
                tensor=Q.tensor, offset=Q[row0 : row0 + 1, :].offset,
                ap=dims,
            )

        nc.sync.dma_start(out=d_rscr[:, :], in_=r_sb[:, :])
        c_engs = (nc.sync, nc.scalar, nc.sync, nc.scalar)
        for q in range(4):
            qsrc = bass.AP(
                tensor=d_rscr, offset=QCOL * q,
                ap=[[1, 1], [0, NB], [1, QCOL]],
            )
            c_engs[q].dma_start(out=Q[32 * q : 32 * q + NB, :], in_=qsrc)

        # ---------- dense radial basis: clamp -> sin -> square ----------
        Qc = single.tile([128, QCOL], F32)
        nc.vector.tensor_scalar(
            out=Qc, in0=Q, scalar1=rlo, scalar2=rhi, op0=OP.max, op1=OP.min,
        )
        Qs = single.tile([128, QCOL], F16)
        nc.scalar.activation(
            out=Qs, in_=Qc, func=AF.Sin, scale=SBASIS, bias=sbias,
        )
        # preload Silu table while the basis is squared
        nc.scalar.activation(out=dum, in_=Qs[0:1, 0:1], func=AF.Silu)
        Qsq = single.tile([128, QCOL], F16)
        nc.vector.tensor_mul(out=Qsq, in0=Qs, in1=Qs)
        if stage == "basis":
            nc.sync.dma_start(out=d_out[:, :], in_=Qsq)
            return

        # ---------- G[h, o*N + b] = sum_i w3[h, o*C+i] * x[b, i] ----------
        # One matmul per w3T chunk: lhsT [128=(2 o's, i), H], rhs = xT2
        # block-diag -> out [H, 128] = [G_{2c} | G_{2c+1}].
        G_sb = single.tile([H, C * N], F16)

        def emit_g(g):
            # four G matmuls into quarters of one PSUM bank (start=True zeroes
            # the whole 2KB zero-region; the rest accumulate onto pending
            # zeros), then a single PSUM->SBUF copy for all 512 cols.
            pg = ps_g.tile([H, 512], F32, tag="g")
            for t in range(4):
                c = 4 * g + t
                nc.tensor.matmul(
                    out=pg[:, 128 * t : 128 * (t + 1)],
                    lhsT=w3T_sb[:, c, :], rhs=xT2,
                    start=(t == 0), stop=(t == 3), skip_group_check=True,
                )
            nc.vector.tensor_copy(
                out=G_sb[:, 512 * g : 512 * (g + 1)], in_=pg)

        for g in range(4):
            emit_g(g)
        if stage == "G":
            for g in range(4, 8):
                emit_g(g)
            nc.sync.dma_start(out=d_out[:, :], in_=G_sb)
            return

        # ---------- fused radial MLP, chunk-pipelined, G interleaved ----------
        h2_sb = single.tile([H, NP], F16)
        ps_y = ps_fin.tile([N, C], F32, tag="y")
        G_v = G_sb[:, :].rearrange("h (o b) -> h b o", b=N)
        ydone = [0]

        def emit_y(b_hi):
            # y[a,o] += h2_b^T @ G_b ; h2[:, b*N:(b+1)*N] works as h2[h, a*N+b]
            # by (a,b) symmetry of r.  rhs[k, n] = G[k, 64*n + b].
            while ydone[0] < b_hi:
                b = ydone[0]
                nc.tensor.matmul(
                    out=ps_y,
                    lhsT=h2_sb[:, b * N : (b + 1) * N],
                    rhs=G_v[:, b, :],
                    start=(b == 0), stop=(b == N - 1),
                )
                ydone[0] += 1

        for j in range(N_CHUNKS if stage != "y" else 0):
            q, half = j // 2, j % 2
            p1 = ps_h1.tile([H, PAIR_CHUNK], F32)
            nc.tensor.matmul(
                out=p1, lhsT=w1_sb[32 * q : 32 * q + NB, :],
                rhs=Qsq[32 * q : 32 * q + NB,
                        half * PAIR_CHUNK : (half + 1) * PAIR_CHUNK],
                start=True, stop=True,
                tile_position=(32 * q, 0),
            )
            h1c = work.tile([H, PAIR_CHUNK], F16)
            nc.scalar.activation(out=h1c, in_=p1, func=AF.Silu, scale=SCALE1)
            p2 = ps_h2.tile([H, PAIR_CHUNK], F32)
            nc.tensor.matmul(out=p2, lhsT=w2_sb, rhs=h1c, start=True, stop=True)
            nc.scalar.activation(
                out=h2_sb[:, j * PAIR_CHUNK : (j + 1) * PAIR_CHUNK],
                in_=p2, func=AF.Silu, scale=SCALE2,
            )
            if j < 4:
                emit_g(4 + j)
            else:
                # all G in SBUF; contract h2 chunks as they appear
                emit_y(8 * j)
        if stage == "h2":
            nc.sync.dma_start(out=d_out[:, :], in_=h2_sb)
            return
        # preload Sqrt table (for the std) behind the tail y matmuls; the
        # h2 dep pins it after the last silu
        nc.scalar.activation(
            out=dum, in_=h2_sb[0:1, NP - 1 : NP], func=AF.Sqrt)
        emit_y(N)

        # ---------- |y| * mask, column-sum over a ----------
        absx = work.tile([N, C], F16)
        nc.scalar.activation(
            out=absx, in_=ps_y, func=AF.Abs, scale=SCALE_FINAL,
        )
        if stage == "y":
            nc.sync.dma_start(out=d_out[:, :], in_=absx)
            return
        nc.vector.tensor_scalar_mul(out=absx, in0=absx, scalar1=mask_f)

        ones64 = single.tile([N, 1], F16)
        nc.vector.memset(ones64, 1.0)
        ps_s = ps_small.tile([1, C], F32, tag="sm")
        nc.tensor.matmul(out=ps_s, lhsT=ones64, rhs=absx, start=True, stop=True)

        # ---------- mean/std(ddof=1) normalize over C ----------
        ssum = work.tile([1, 1], F32)
        nc.vector.reduce_sum(out=ssum, in_=ps_s, axis=mybir.AxisListType.X)
        m_s = work.tile([1, 1], F32)
        nc.vector.tensor_scalar_mul(out=m_s, in0=ssum, scalar1=1.0 / C)
        d_row = work.tile([1, C], F32)
        nc.vector.tensor_scalar(
            out=d_row, in0=ps_s, scalar1=m_s, scalar2=None, op0=OP.subtract,
        )
        dsq = work.tile([1, C], F32)
        nc.vector.tensor_mul(out=dsq, in0=d_row, in1=d_row)
        qsum = work.tile([1, 1], F32)
        nc.vector.reduce_sum(out=qsum, in_=dsq, axis=mybir.AxisListType.X)
        stddev = work.tile([1, 1], F32)
        nc.scalar.activation(
            out=stddev, in_=qsum, func=AF.Sqrt, scale=1.0 / (C - 1),
        )
        nc.vector.tensor_scalar_add(out=stddev, in0=stddev, scalar1=EPS)
        rec = work.tile([1, 1], F32)
        nc.vector.reciprocal(out=rec, in_=stddev)
        norm_row = work.tile([1, C], F32)
        nc.vector.tensor_scalar_mul(out=norm_row, in0=d_row, scalar1=rec)

        # ---------- head: leaky_relu(norm @ fc3 / 8) @ fc2 / 8 -> sigmoid ----
        ident1 = single.tile([1, 1], F32)
        nc.vector.memset(ident1, 1.0)
        ps_nT = ps_small.tile([C, 1], F32, tag="sm")
        nc.tensor.transpose(out=ps_nT, in_=norm_row, identity=ident1)
        normT = work.tile([C, 1], F32)
        nc.vector.tensor_copy(out=normT, in_=ps_nT)

        ps_y1 = ps_small.tile([C, 1], F32, tag="sm")
        nc.tensor.matmul(out=ps_y1, lhsT=fc3_sb, rhs=normT, start=True, stop=True)
        y1_sb = work.tile([C, 1], F32)
        y1_neg = work.tile([C, 1], F32)
        nc.vector.tensor_scalar_mul(out=y1_sb, in0=ps_y1, scalar1=FC_SCALE)
        nc.vector.tensor_scalar_mul(
            out=y1_neg, in0=ps_y1, scalar1=FC_SCALE * LRELU_ALPHA)
        nc.vector.tensor_tensor(out=y1_sb, in0=y1_sb, in1=y1_neg, op=OP.max)

        ps_y2 = ps_small.tile([1, 1], F32, tag="sm")
        nc.tensor.matmul(out=ps_y2, lhsT=y1_sb, rhs=fc2_sb, start=True, stop=True)
        res = work.tile([1, 1], F32)
        nc.scalar.activation(
            out=res, in_=ps_y2, func=AF.Sigmoid, scale=FC_SCALE,
        )
        nc.sync.dma_start(out=d_out[:, :], in_=res)

    with tile.TileContext(nc) as tc:
        with (
            tc.tile_pool(name="single", bufs=1) as single,
            tc.tile_pool(name="work", bufs=2) as work,
            tc.tile_pool(name="ps_h1", bufs=2, space="PSUM") as ps_h1,
            tc.tile_pool(name="ps_h2", bufs=2, space="PSUM") as ps_h2,
            tc.tile_pool(name="ps_g", bufs=2, space="PSUM") as ps_g,
            tc.tile_pool(name="ps_fin", bufs=1, space="PSUM") as ps_fin,
            tc.tile_pool(name="ps_small", bufs=1, space="PSUM") as ps_small,
        ):
            _emit(tc, single, work, ps_h1, ps_h2, ps_g, ps_fin, ps_small)
    nc.finalize()
    return nc


_NC_CACHE = None


def _get_nc():
    global _NC_CACHE
    if _NC_CACHE is None:
        _NC_CACHE = _build_nc()
    return _NC_CACHE


def kernel(**inputs) -> np.ndarray:
    nc = _get_nc()
    # pure relayout of w3 (done once on host): w3T[p, c, h] = w3[h, c*128+p]
    w3 = np.asarray(inputs["w3"], dtype=np.float32)
    w3T = np.ascontiguousarray(
        w3.reshape(H, C * C // 128, 128).transpose(2, 1, 0).reshape(128, -1)
    ).astype(np.float16)
    w1q = np.zeros((128, H), np.float16)
    for q in range(4):
        w1q[32 * q : 32 * q + NB, :] = np.asarray(inputs["w1"], np.float16)
    in_maps = []
    for z in range(N_CORES):
        xT = np.concatenate(
            [inputs["input1"][z], inputs["input2"][z]], axis=0).astype(np.float32).T
        xT2 = np.zeros((128, 128), np.float16)
        xT2[0:64, 0:64] = xT
        xT2[64:128, 64:128] = xT
        xyz = np.concatenate(
            [inputs["xyz1"][z], inputs["xyz2"][z]], axis=0).astype(np.float32)
        in_maps.append({
            "xT2": xT2,
            "xyzT": np.ascontiguousarray(xyz.T),
            "xyz": np.ascontiguousarray(xyz),
            "mask": np.ascontiguousarray(
                inputs["mask"][z].reshape(N, 1), dtype=np.int32),
            "w1": w1q,
            "w2": np.ascontiguousarray(inputs["w2"], dtype=np.float16),
            "w3T": w3T,
            "fc3_w": np.ascontiguousarray(inputs["fc3_w"], dtype=np.float32),
            "fc2_w": np.ascontiguousarray(inputs["fc2_w"], dtype=np.float32),
        })
    out = run_bass_kernel_spmd(nc, in_maps, core_ids=list(range(N_CORES)))
    return np.concatenate(
        [r["out"].reshape(-1) for r in out.results]).astype(np.float32)
